# revision 3
# baseline (speedup 1.0000x reference)
"""Trainium2 Bass kernel for nn_Dense_EI (dense EI-masked MLP layer).

Math: out = scale * concat([x_exc, -4*x_inh], -1) @ bool_kernel
    = x @ K'  where K' = scale * kernel with inhibitory rows scaled by -4.

Device strategy:
  - Fold the EI column flip and the output scale into the (4096, 4096) bool
    kernel on host; resulting entries {0, s, -4s} are exact in bf16.
  - Split x into bf16 hi + lo parts (x ~= hi + lo), stack them along the
    contraction dim, and duplicate K' along the contraction dim. One bf16
    matmul with K=8192 then produces a near-fp32-accurate result
    (PSUM accumulates in fp32; measured rel err ~2.5e-6).
  - Data-parallel over the 8 NeuronCores: each core computes 1024 of the
    8192 output rows: out_c = kxm_c.T @ kxn, with kxm_c = (8192, 1024)
    (transposed x hi/lo shard) and kxn = (8192, 4096).
"""

import sys

if "/opt/trn_rl_repo" not in sys.path:
    sys.path.insert(0, "/opt/trn_rl_repo")

import ml_dtypes
import numpy as np

N_CORES = 8
B, S, IN_DIM, FEATURES = 4, 2048, 4096, 4096
M_TOTAL = B * S
P_EXC = 0.8
NUM_EXC = round(IN_DIM * P_EXC)  # 3277

_module_cache: dict = {}


def _install_neff_cache():
    """Cache compiled NEFFs by BIR content hash (walrus compile is ~minutes)."""
    import hashlib
    import os
    import shutil

    from concourse import bass2jax

    if getattr(bass2jax, "_ant_neff_cache_installed", False):
        return
    orig = bass2jax.compile_bir_kernel
    cache_dir = os.environ.get("BASS_NEFF_CACHE", "/tmp/bass_neff_cache")
    os.makedirs(cache_dir, exist_ok=True)

    def cached(bir_json, tmpdir, neff_name="file.neff"):
        data = bir_json if isinstance(bir_json, bytes) else bir_json.encode()
        h = hashlib.sha256(data).hexdigest()[:24]
        path = os.path.join(cache_dir, f"{h}.neff")
        if os.path.exists(path):
            dst = os.path.join(tmpdir, neff_name)
            shutil.copy(path, dst)
            return dst
        out = orig(bir_json, tmpdir, neff_name=neff_name)
        shutil.copy(out, path + f".tmp{os.getpid()}")
        os.replace(path + f".tmp{os.getpid()}", path)
        return out

    bass2jax.compile_bir_kernel = cached
    bass2jax._ant_neff_cache_installed = True


def _build_module(k2: int, m_core: int, n: int):
    """Build + compile the per-core Bass module: mxn = kxm.T @ kxn."""
    key = (k2, m_core, n)
    if key in _module_cache:
        return _module_cache[key]

    import concourse.bacc as bacc
    import concourse.mybir as mybir
    import concourse.tile as tile
    from concourse.kernels.tile_matmul import matmul_tile_kernel

    nc = bacc.Bacc("TRN2", target_bir_lowering=False, debug=False)
    kxm = nc.dram_tensor("kxm", [k2, m_core], mybir.dt.bfloat16, kind="ExternalInput")
    kxn = nc.dram_tensor("kxn", [k2, n], mybir.dt.bfloat16, kind="ExternalInput")
    mxn = nc.dram_tensor("mxn", [m_core, n], mybir.dt.float32, kind="ExternalOutput")
    with tile.TileContext(nc) as tc:
        matmul_tile_kernel(tc, kxm.ap(), kxn.ap(), mxn.ap())
    nc.compile()
    _module_cache[key] = nc
    return nc


def _prep_inputs(x_np: np.ndarray, kern_np: np.ndarray, scale_np: np.ndarray):
    """Host-side: EI/scale fold into kernel, hi/lo split of x, per-core shards."""
    in_dim = kern_np.shape[0]
    num_exc = round(in_dim * P_EXC)
    m_total = x_np.size // in_dim

    kf = kern_np.astype(np.float32)
    ei = np.float32(-P_EXC / (1.0 - P_EXC))  # == -4.0 exactly in f32
    kf[num_exc:] *= ei
    kf *= np.float32(scale_np)
    kf_bf = kf.astype(ml_dtypes.bfloat16)
    kxn = np.ascontiguousarray(np.concatenate([kf_bf, kf_bf], axis=0))

    xs = x_np.reshape(m_total, in_dim)
    x_hi = xs.astype(ml_dtypes.bfloat16)
    x_lo = (xs - x_hi.astype(np.float32)).astype(ml_dtypes.bfloat16)
    # (2*in_dim, m_total): rows 0..in_dim-1 = hi.T, rows in_dim.. = lo.T
    kxm_full = np.ascontiguousarray(
        np.concatenate([x_hi.T, x_lo.T], axis=0)
    )

    m_core = m_total // N_CORES
    kxm_shards = [
        np.ascontiguousarray(kxm_full[:, c * m_core : (c + 1) * m_core])
        for c in range(N_CORES)
    ]
    return kxm_shards, kxn, m_core


def _run(x_np, kern_np, scale_np, trace=False, tmpdir=None):
    from concourse.bass_utils import run_bass_kernel_spmd

    _install_neff_cache()

    kxm_shards, kxn, m_core = _prep_inputs(x_np, kern_np, scale_np)
    k2, n = kxn.shape
    nc = _build_module(k2, m_core, n)

    in_maps = [{"kxm": kxm_shards[c], "kxn": kxn} for c in range(N_CORES)]
    res = run_bass_kernel_spmd(
        nc, in_maps, list(range(N_CORES)), trace=trace, tmpdir=tmpdir
    )
    out = np.concatenate([res.results[c]["mxn"] for c in range(N_CORES)], axis=0)
    return out, res


def kernel(x, kernel, scale):
    x_np = np.asarray(x, dtype=np.float32)
    kern_np = np.asarray(kernel)
    scale_np = np.asarray(scale, dtype=np.float32)
    out, _ = _run(x_np, kern_np, scale_np)
    return out.reshape(x_np.shape[:-1] + (kern_np.shape[1],))
